# revision 34
# baseline (speedup 1.0000x reference)
"""Soft-DTW-style loss (soft-min of pairwise sq-distances) on Trainium2.

Data-parallel over batch B=8: one batch element per NeuronCore. Per core
a bf16 matmul produces u[i,j] = 127 - d[i,j]*log2e directly in PSUM via
12 augmented contraction rows:
  rows 0-7 : p_f (bf16)          x  2*log2e*t_f (bf16)
  rows 8-9 : 1.0                 x  A_hi, A_lo   (A = 127 - log2e*t2)
  rows 10-11: B_hi, B_lo         x  -1.0         (B = log2e*p2)
The hi/lo bf16 splits keep the large per-row/col constants exact to
~5e-4; the remaining feature-product noise is ~1% zero-mean, which
averages out in the final mean over 16.8M soft-mins.

The exp + row-sum alternates whole (128, 2048) tiles between engines:
  Scalar ACT (10 tiles): exp via LUT with accum_out row-sum. Exact.
  Vector DVE (6 tiles):
    pass1 (tensor_scalar, PSUM->SBUF): i32 = int32(max(u,0) * 2^23);
      bitcast(i32) = 2^(k-127) * (1+f)  (Schraudolph exp2 approx)
    pass2 (tensor_scalar from SBUF + accum_out): row-sum of the bitcast
      values (single-src SBUF op -> eligible for 2x_2p DVE perf mode).
  Host multiplies vector-tile sums by CS_V = 1/E_f[(1+f)*2^-f]; the
  residual +-3% per-element error is zero-mean (loss err ~1e-4).

Whole-tile alternation pays per-instruction overhead (ACT 352-cycle pipe
+ accumulator read, DVE dispatch) once per 2048 columns and keeps both
PSUM read ports streaming. Inputs are one packed bf16 DRAM blob per PE
row-group (pred weights + targ chunk), 4 descriptors over the fast DGE
queues (scalar HWDGE + gpsimd SWDGE; the SP queue is slow).

Self-contained: hardcodes shapes B=8, L=2048, F=8.
"""

import numpy as np
from contextlib import ExitStack

B, L, F = 8, 2048, 8
P = 128          # partition tile height (i rows per tile)
NT = L // P      # 16 i-tiles
KB = 12          # augmented contraction rows (bf16 with hi/lo splits)
JC = 512         # j-chunk (one PE row-group each)
NJ = L // JC     # 4 j-chunks
WCOL = L + JC    # packed input blob columns: 2048 pred + 512 targ chunk

LOG2E = 1.4426950408889634
LN2 = 0.6931471805599453
# 1 / integral_0^1 (1+f) 2^-f df : host-side rescale of the Schraudolph sums
CS_V = 0.9609060278364027

# vector-consumed tiles (pattern s,s,v,s,v -> 10 scalar / 6 vector).
# Vector tiles stay spaced >=2 apart: adjacent vector tiles serialize
# pass1+pass2 pairs on the engine (~7us bursts) and starve scalar past
# its 2-PSUM-buffer lookahead. The last tile stays scalar so the output
# DMA isn't gated by vector's longer per-tile path.
VTILES = (2, 4, 7, 9, 12, 14)
STILES = tuple(t for t in range(NT) if t not in VTILES)

_cache = {}


def _register_sum2x():
    """Row-sum custom DVE op with a hand-written 2X_2PORT uop program.

    1x semantics: out = in0 (trash), accum_out = row sum. The 2x_2p variant
    reads two consecutive fp32 elements per cycle (rd0+rd1, SBUF only),
    pair-adds them at stage 0 and accumulates at stage 1 -- halving the
    per-element cost of the row-sum pass vs the native 1x reduce.
    """
    import copy as _copy
    from operator import add

    from concourse.dve_ops import (
        _COMPILE_CACHE,
        _SUB_OPCODE_FOR_NAME,
        CUSTOM_DVE_SPECS,
        OPS,
        DveOp,
    )
    from concourse.dve_spec import Spec, Src0, lower
    from concourse.dve_uop import (
        ENABLE,
        AluInp,
        AluOp,
        DveOpSpec,
        OutPath,
        OutSel,
        InpSel,
        Trigger,
        UopConfig,
    )

    NAME = "SUM_PAIR2X_ANT"
    if NAME in _SUB_OPCODE_FOR_NAME:
        return next(op for op in OPS if op.name == NAME)

    def _ref(in0, in1, c0, c1, c2):
        x = np.ascontiguousarray(in0, np.float32)
        acc = x.reshape(x.shape[0], -1).sum(axis=-1, keepdims=True)
        return x, acc

    spec = Spec(body=Src0, accum=add, reference=_ref)
    uops_1x = lower(spec, ver="v3")  # [seed, steady]
    assert len(uops_1x) == 2

    # 2X_1PORT slot filler: requires 2-byte dtype so it never engages for
    # the fp32 call sites; reuse the 1x program to populate the slot.
    uops_2x = [_copy.deepcopy(u) for u in uops_1x]

    # 2X_2PORT steady state, mirroring the stock TENSOR_SCALAR slot-18
    # conventions: even element = SRC_0 on lane 0 (block0 PREV_ALU_OUT),
    # odd element = SRC_1 on lane 3 (block0 PREV_DELAY_2), BOTH
    # requires_src0 and requires_src1 set, two writes per cycle.
    steady = UopConfig()
    steady.enable_input(InpSel.SRC_0, 0)  # even element -> block0 mux
    steady.enable_input(InpSel.SRC_1, 3)  # odd element -> delay chain 2
    steady.datapath_config[0].enable_alu(
        AluOp.ADD, AluInp.PREV_ALU_OUT, AluInp.PREV_DELAY_2
    )
    steady.datapath_config[1].enable_alu(
        AluOp.ADD, AluInp.CURR_ALU_OUT, AluInp.PREV_ALU_OUT
    )
    for st in range(1, 8):
        if st >= 2:
            steady.datapath_config[st].pass_through_alu()
        steady.datapath_config[st].alu_out_a_enable = ENABLE
    steady.accum_enabled = ENABLE
    steady.require_inp0 = ENABLE
    steady.require_inp1 = ENABLE
    steady.trigger = (Trigger.SRC_TENSOR_DONE, Trigger.NONE, Trigger.NONE)
    steady.next_uop = (0, 0, 0)
    # two writes/cycle of the running accum keeps the write count equal to
    # the element count (stock 2P convention); the out tensor is scratch
    steady.enable_output(OutSel.ALU_OUT, OutPath.WR0_LO)
    steady.enable_output(OutSel.ALU_OUT, OutPath.WR1_LO)
    uops_2x2p = [_copy.deepcopy(uops_1x[0]), steady]

    used_rows = set(_SUB_OPCODE_FOR_NAME.values())
    row = next(r for r in range(1, 0x20) if r not in used_rows)
    _SUB_OPCODE_FOR_NAME[NAME] = row

    compiled = DveOpSpec(
        name=NAME,
        opcode=row,
        uops=uops_1x,
        uops_2x=uops_2x,
        uops_2x_2p=uops_2x2p,
        perf_max=2,
        rd1_en=False,
    )
    compiled.validate("v3")
    op = DveOp(NAME, spec, subdim=False, uops_sha={"v3": compiled.sha("v3")})
    OPS.append(op)
    CUSTOM_DVE_SPECS[NAME] = spec
    # compile() consults this cache first, bypassing the lower()-based
    # reconstruction (which would drop the perf variants)
    _COMPILE_CACHE[(NAME, "v3")] = compiled
    return op


def _emit_sum2x(nc, op, out, in0, accum_out):
    """Emit the SUM_PAIR2X_ANT instruction with perf_max=2 (2x_2p reachable).

    Mirrors BassVectorEngine._custom_dve, which hardcodes perf_max=0.
    """
    from concourse import bass_isa, mybir
    from concourse.dve_ops import get_dve_sub_opcode

    v = nc.vector
    if op.name not in nc.m.ant_custom_dve_ops:
        nc.m.ant_custom_dve_ops = sorted({*nc.m.ant_custom_dve_ops, op.name})
    shape = bass_isa.CustomDveShape.TTSS
    isa_opcode = nc.isa.Opcode[
        f"NEURON_ISA_TPB_OPCODE_CUSTOM_DVE_ANT_{shape.slot()}"
    ].value
    imm = mybir.ImmediateValue(dtype=mybir.dt.float32, value=0.0)
    return v.add_instruction(
        bass_isa.InstCustomDveAnt(
            name=nc.get_next_instruction_name(),
            op_name=op.name,
            rd1_en=False,
            subdim=0,
            imm2=0.0,
            shape=shape,
            row=get_dve_sub_opcode(op.name),
            isa_opcode=isa_opcode,
            perf_max=2,
            ins=[v.lower_ap(in0, for_isa=True), imm, imm],
            outs=[
                v.lower_ap(out, for_isa=True),
                v.lower_ap(accum_out, for_isa=True),
            ],
        )
    )


def _build_nc():
    import concourse.tile as tile
    from concourse import bacc, mybir

    sum2x = _register_sum2x()

    dtf = mybir.dt.float32
    dtb = mybir.dt.bfloat16
    dti = mybir.dt.int32
    nc = bacc.Bacc("TRN2", target_bir_lowering=False, debug=False, num_devices=B)
    pt = nc.dram_tensor("pt", [4 * KB, WCOL], dtb, kind="ExternalInput").ap()
    s_out = nc.dram_tensor("s_out", [P, NT], dtf, kind="ExternalOutput").ap()

    NS, NV = len(STILES), len(VTILES)

    with tile.TileContext(nc) as tc, ExitStack() as ctx:
        sb = ctx.enter_context(tc.tile_pool(name="sb", bufs=1))

        # packed operands: per row-group q, partitions 32q..32q+KB hold
        # pred weights (cols 0:2048) and the targ j-chunk (cols 2048:2560)
        inAT = sb.tile([128, WCOL], dtb)
        S_all = sb.tile([P, NT], dtf)  # cols [0:NS) scalar, [NS:) vector
        bias_c = sb.tile([P, 1], dtf)  # ACT bias: -127*ln2
        nc.gpsimd.memset(bias_c[:], -127.0 * LN2)

        # 4 input descriptors, one per DGE lane (a second gpsimd descriptor
        # spills to the slow SP queue, so sync takes the 4th directly)
        nc.gpsimd.dma_start(inAT[64 : 64 + KB, :], pt[2 * KB : 3 * KB, :])
        nc.scalar.dma_start(inAT[0:KB, :], pt[0:KB, :])
        nc.scalar.dma_start(inAT[32 : 32 + KB, :], pt[KB : 2 * KB, :])
        nc.sync.dma_start(inAT[96 : 96 + KB, :], pt[3 * KB : 4 * KB, :])

        scratch = ctx.enter_context(tc.tile_pool(name="scr", bufs=2))

        # Pre-load the exp ACT table set (~1.3us) while DMAs are in flight:
        # walrus inserts the PSEUDO_LOAD before this dependency-free dummy.
        dummy = scratch.tile([P, 1], dtf, tag="dummy")
        nc.scalar.activation(
            dummy[:], bias_c[:, 0:1], mybir.ActivationFunctionType.Exp,
            bias=bias_c[:, 0:1], scale=1.0,
        )

        i_s = i_v = 0
        with tc.tile_pool(name="pm", bufs=2, space="PSUM") as pm:
            for t in range(NT):
                ptp = pm.tile([P, L], dtf, tag="ptp")  # 4 PSUM banks
                for q in range(NJ):
                    nc.tensor.matmul(
                        ptp[:, q * JC : (q + 1) * JC],
                        inAT[32 * q : 32 * q + KB, t * P : (t + 1) * P],
                        inAT[32 * q : 32 * q + KB, L:WCOL],
                        start=True,
                        stop=True,
                        # explicit: base_partition() auto-derive rejects 96
                        tile_position=(32 * q, 0),
                    )
                if t in VTILES:
                    # Vector pass1: int32(max(u,0) * 2^23) -> SBUF
                    xI = scratch.tile([P, L], dtf, tag="xI")
                    nc.vector.tensor_scalar(
                        xI.bitcast(dti)[:],
                        ptp[:],
                        0.0,
                        float(2.0**23),
                        mybir.AluOpType.max,
                        mybir.AluOpType.mult,
                    )
                    # Vector pass2: row-sum of the bitcast values at 2
                    # elem/cycle (custom 2x_2p pair-sum op, SBUF src)
                    eV = scratch.tile([P, L], dtf, tag="eV")
                    _emit_sum2x(
                        nc, sum2x, out=eV[:], in0=xI[:],
                        accum_out=S_all[:, NS + i_v : NS + i_v + 1],
                    )
                    i_v += 1
                else:
                    # Scalar: LUT exp over the whole tile + accum row-sum
                    eT = scratch.tile([P, L], dtb, tag="eT")
                    nc.scalar.activation(
                        eT[:],
                        ptp[:],
                        mybir.ActivationFunctionType.Exp,
                        bias=bias_c[:, 0:1],
                        scale=LN2,
                        accum_out=S_all[:, i_s : i_s + 1],
                    )
                    i_s += 1

        # One fused output descriptor on scalar's HWDGE queue: it waits on
        # the later of scalar's final accumulator read (program order) and
        # vector's (cross-engine dep), saving a second ~0.7us issue.
        nc.scalar.dma_start(s_out[:], S_all[:])

    nc.compile()
    return nc


def get_nc():
    if "nc" not in _cache:
        _cache["nc"] = _build_nc()
    return _cache["nc"]


def host_prep(pred_b: np.ndarray, target_b: np.ndarray) -> dict:
    """Pack one batch element into the bf16 device input layout."""
    import ml_dtypes

    bf = ml_dtypes.bfloat16
    pred_b = np.ascontiguousarray(pred_b, dtype=np.float64)
    target_b = np.ascontiguousarray(target_b, dtype=np.float64)

    p2 = np.sum(pred_b * pred_b, axis=1)
    t2 = np.sum(target_b * target_b, axis=1)
    pa = np.zeros((KB, L), bf)
    pa[:F] = pred_b.T.astype(bf)
    pa[F] = 1.0
    pa[F + 1] = 1.0
    Bv = LOG2E * p2
    Bh = Bv.astype(bf)
    pa[F + 2] = Bh
    pa[F + 3] = (Bv - Bh.astype(np.float64)).astype(bf)
    ta = np.zeros((KB, L), bf)
    ta[:F] = (2.0 * LOG2E * target_b.T).astype(bf)
    A = 127.0 - LOG2E * t2
    Ah = A.astype(bf)
    ta[F] = Ah
    ta[F + 1] = (A - Ah.astype(np.float64)).astype(bf)
    ta[F + 2] = -1.0
    ta[F + 3] = -1.0
    pt = np.empty((4 * KB, WCOL), bf)
    for q in range(NJ):
        pt[q * KB : (q + 1) * KB, :L] = pa
        pt[q * KB : (q + 1) * KB, L:] = ta[:, q * JC : (q + 1) * JC]
    return {"pt": np.ascontiguousarray(pt)}


def combine_s(s_out_b: np.ndarray) -> np.ndarray:
    """(128, 16) device output -> (128, 16) total row sums by tile (fp64)."""
    s = s_out_b.astype(np.float64)
    out = np.empty((P, NT), np.float64)
    for i, t in enumerate(STILES):
        out[:, t] = s[:, i]
    for i, t in enumerate(VTILES):
        out[:, t] = CS_V * s[:, len(STILES) + i]
    return out


def reduce_host(s_stack: np.ndarray) -> np.ndarray:
    """(B, 128, 16) raw outputs -> scalar mean(-log S), fp64 accumulate."""
    S = np.stack([combine_s(s_stack[b]) for b in range(B)])
    loss = -np.log(S)
    return np.asarray(loss.mean(), dtype=np.float32)


def run_on_hw(pred: np.ndarray, target: np.ndarray, trace: bool = False):
    from concourse import bass_utils

    nc = get_nc()
    in_maps = [host_prep(pred[b], target[b]) for b in range(B)]
    res = bass_utils.run_bass_kernel_spmd(
        nc, in_maps, core_ids=list(range(B)), trace=trace
    )
    s_stack = np.stack([r["s_out"] for r in res.results])  # (B, 128, 16)
    return reduce_host(s_stack), res


def kernel(pred: np.ndarray, target: np.ndarray) -> np.ndarray:
    pred = np.asarray(pred, dtype=np.float32)
    target = np.asarray(target, dtype=np.float32)
    assert pred.shape == (B, L, F) and target.shape == (B, L, F)
    loss, _ = run_on_hw(pred, target)
    return loss
